# revision 72
# baseline (speedup 1.0000x reference)
"""Trainium2 Bass kernel for nn_Attention_11055245820093.

Swin-style attention block: qkv proj -> per-head scaled dot-product attention
with 2D relative position bias (CLS zero-padded), per-head softplus temperature,
patch-diagonal mask -> proj.

Strategy: data-parallel over batch B=64 across 8 NeuronCores (8 batches/core).
All compute per core runs in a "transposed" layout (channels on partitions,
tokens on the free dim) so no on-device transposes are needed:
  - QK^T projection computed in out^T (c, t) layout with fp8e4m3 DoubleRow
    matmuls (2 k-tiles per instruction, 4x bf16 throughput) using
    error-compensated hi+lo operand pairs (hi*hi "main" terms pair two
    k-tiles; "cross" terms compute W_hi^T x_lo + W_lo^T x_hi), so accuracy
    matches bf16.  The dual-fp8 Ldweights path requires the two k-sub-tiles
    of the stationary operand to be contiguous, hence the blocked host
    layouts.  Weights are host-prescaled (SQ/SK) out of the fp8 subnormal
    range; undone by the ACT copy's scale.
  - V computed in (t, c) layout directly (lhsT = x^T, bf16), with a ones
    column appended per head so the AV matmul also produces the softmax
    denominators (row 64 of the PSUM output) for free
  - S^T(j,i) = K^T.T @ Q^T per (batch,head); scale/temp folded into the q
    weights; rel-pos bias/mask applied multiplicatively via a host-exp'd
    table.  The two heads of a pair occupy the two banks of one 2-bank PSUM
    tile (each matmul its own accumulation group -- the HW rejects multiple
    groups packed into one bank) so exp/bias-mul run once per (pair, jtile).
  - softmax normalization: reciprocal of the denominators row on DVE
    (GPSIMD cannot read PSUM), broadcast to all partitions with a GPSIMD
    partition_broadcast, then the divide runs as two DVE multiplies (head1
    partition-shifts to rows 64..127)
  - proj consumes attn_out^T directly; v-bias and proj bias fold into a
    host-side constant added after gather.
"""

import os
import sys

sys.path.insert(0, "/opt/trn_rl_repo")
os.environ.setdefault("MYCRO_LOCAL_CACHE", "1")

import numpy as np
import ml_dtypes

BF16 = ml_dtypes.bfloat16

# Problem constants (hardcoded per contract)
B, N, C, H, D = 64, 197, 768, 12, 64
NCORES = 8
BPC = B // NCORES          # 8 batches per core
T = BPC * N                # 1576 tokens per core
KT = C // 128              # 6 contraction tiles of 128
NT = 4                     # token n-tiles
TN = T // NT               # 394 tokens per n-tile
SCALE = D ** -0.5

_CACHE = {}

TRACE = False
LAST_RESULTS = None


def _build(finalize=True):
    import concourse.bass as bass
    import concourse.tile as tile
    from concourse import bacc, library_config, mybir

    dt = mybir.dt
    f32, bf16, f8 = dt.float32, dt.bfloat16, dt.float8e4
    AF = mybir.ActivationFunctionType
    OP = mybir.AluOpType
    DR = mybir.MatmulPerfMode.DoubleRow

    nc = bacc.Bacc("TRN2", target_bir_lowering=False, debug=False)

    # Phase A (QK proj) runs in fp8 DoubleRow with error-compensated hi/lo
    # pairs, pre-split on host.  HW dual-fp8 Ldweights requires the two
    # k-sub-tiles CONTIGUOUS, so weights are stored as [.., mt, unit, 2, 128]
    # blocks and x as [nt, .., unit, 2, TN] flat pairs.  "hi" units pair
    # (hi_k0, hi_k1); "x"/cross units pair weights (hi_k, lo_k) against
    # moving (lo_k, hi_k) so one DoubleRow computes W_hi^T x_lo + W_lo^T x_hi.
    # Phase B (V) stays bf16: its stationary operand is x sliced by token
    # window, which cannot be made pair-contiguous without duplicating x.
    # x stored once, window-blocked + zero-padded to 128 per window, in
    # memory window order [w0, w2, w1, w3] (the two 128-token windows first),
    # so Phase B gets pair-contiguous stationary slices and Phase A reads
    # window GROUPS as contiguous psum column blocks.
    x_hi = nc.dram_tensor(
        "x_hi", [NT, 128, 4, KT // 2, 2, 128], f8, kind="ExternalInput"
    ).ap()
    x_x = nc.dram_tensor(
        "x_x", [NT, 128, 4, KT, 2, 128], f8, kind="ExternalInput"
    ).ap()
    wv_x = nc.dram_tensor(
        "wv_x", [2, 128, KT, 2, C // 2], f8, kind="ExternalInput"
    ).ap()
    wqk_hi = nc.dram_tensor(
        "wqk_hi", [128, 2 * KT, KT // 2, 2, 128], f8, kind="ExternalInput"
    ).ap()
    wqk_x = nc.dram_tensor(
        "wqk_x", [128, 2 * KT, KT, 2, 128], f8, kind="ExternalInput"
    ).ap()
    wpj = nc.dram_tensor("wpj", [KT, 128, C], bf16, kind="ExternalInput").ap()
    bT = nc.dram_tensor("bT", [KT, N, 2 * N], bf16, kind="ExternalInput").ap()
    bqk = nc.dram_tensor("bqk", [128, 2 * KT], f32, kind="ExternalInput").ap()
    outT = nc.dram_tensor("outT", [KT, 128, T], f32, kind="ExternalOutput").ap()

    JROWS = (128, N - 128)  # 128, 69
    SVF = 64.0              # wv host-scale; ones column matches so the
                            # softmax divide cancels it exactly
    N2 = 2 * N              # 394

    with tile.TileContext(nc) as tc:
        from contextlib import ExitStack

        with ExitStack() as ctx:
            nc.gpsimd.load_library(library_config.attn)
            cp = ctx.enter_context(tc.tile_pool(name="consts", bufs=1))
            psA = ctx.enter_context(tc.tile_pool(name="psA", bufs=2, space="PSUM"))
            psC = ctx.enter_context(tc.tile_pool(name="psC", bufs=3, space="PSUM"))
            wp = ctx.enter_context(tc.tile_pool(name="work", bufs=2))

            # ---- persistent SBUF tiles; DMAs in consumption order ----
            # fp8 x (window-blocked) and wv arrive first (Phase B needs
            # them); x is chunked by nt so batch-0 compute starts early.
            # Startup-critical DMA issues spread across idle queues (SP
            # issue alone costs ~650ns each and serializes the start).
            xh_sb = cp.tile([128, NT, 4, KT // 2, 2, 128], f8, name="xh", tag="xh")
            xx_sb = cp.tile([128, NT, 4, KT, 2, 128], f8, name="xx", tag="xx")
            wvx_sb = cp.tile(
                [128, 2, KT, 2, C // 2], f8, name="wvx", tag="wvx"
            )
            nc.gpsimd.dma_start(out=wvx_sb[:, 0], in_=wv_x[0])
            for nt in range(NT):
                (nc.scalar if nt == 0 else nc.sync).dma_start(
                    out=xh_sb[:, nt], in_=x_hi[nt]
                )
                nc.sync.dma_start(out=xx_sb[:, nt], in_=x_x[nt])
                if nt == 0:
                    nc.gpsimd.dma_start(out=wvx_sb[:, 1], in_=wv_x[1])
            wqkh_sb = cp.tile(
                [128, 2 * KT, KT // 2, 2, 128], f8, name="wqkh", tag="wqkh"
            )
            wqkx_sb = cp.tile(
                [128, 2 * KT, KT, 2, 128], f8, name="wqkx", tag="wqkx"
            )
            nc.sync.dma_start(out=wqkh_sb[:], in_=wqk_hi[:])
            nc.sync.dma_start(out=wqkx_sb[:], in_=wqk_x[:])
            bqk_sb = cp.tile([128, 2 * KT], f32, name="bqk", tag="bqk")
            nc.sync.dma_start(out=bqk_sb[:], in_=bqk[:])
            # bias (hp, jt): both heads of the pair side by side (rows, 2N);
            # loads deferred into the hp loop. proj weights load before D.
            bias_sb = {}
            for hp in range(KT):
                for jt, rows in enumerate(JROWS):
                    bias_sb[(hp, jt)] = cp.tile(
                        [rows, N2], bf16, name=f"bias{hp}_{jt}", tag=f"bias{hp}_{jt}"
                    )
            wpj_sb = [
                cp.tile([128, C], bf16, name=f"wpj{k}", tag=f"wpj{k}")
                for k in range(KT)
            ]

            # qk_sb[0:6] = Q^T tiles (c=0..767), qk_sb[6:12] = K^T tiles
            qk_sb = [
                cp.tile([128, T], bf16, name=f"qk{m}", tag=f"qk{m}")
                for m in range(2 * KT)
            ]
            # V per (batch, jt): (rows, 12 heads, 65) -- 64 V cols + ones col
            v_sb = {}
            for b in range(BPC):
                for jt, rows in enumerate(JROWS):
                    t_ = cp.tile(
                        [rows, H, D + 1], bf16, name=f"v{b}_{jt}", tag=f"v{b}_{jt}"
                    )
                    nc.vector.memset(t_[:, :, D : D + 1], SVF)
                    v_sb[(b, jt)] = t_
            attn_sb = [
                cp.tile([128, T], bf16, name=f"at{m}", tag=f"at{m}") for m in range(KT)
            ]

            # ---- Phase B: V in (t, c) layout, fp8 DoubleRow hi/lo ----
            # stationary = x window blocks (pair-contiguous, zero-padded to
            # 128 rows; pad rows land beyond [0:rows] and are never copied)
            for b in range(BPC):
                ntb = b // 2
                for jt, rows in enumerate(JROWS):
                    wpos = 2 * jt + (b % 2)
                    for n2 in range(2):
                        psv = psA.tile([128, 512], f32, tag="psA")
                        for p in range(KT // 2):
                            nc.tensor.matmul(
                                psv[0:128, 0 : C // 2],
                                xh_sb[:, ntb, wpos, p, :, :],
                                wvx_sb[:, n2, 2 * p : 2 * p + 2, 1, :],
                                start=(p == 0),
                                stop=False,
                                perf_mode=DR,
                            )
                        for k in range(KT):
                            nc.tensor.matmul(
                                psv[0:128, 0 : C // 2],
                                xx_sb[:, ntb, wpos, k, :, :],
                                wvx_sb[:, n2, k, :, :],
                                start=False,
                                stop=(k == KT - 1),
                                perf_mode=DR,
                            )
                        # v-bias folds into the host-side output constant;
                        # copies alternate DVE/ACT to balance engine load
                        if n2 == 0:
                            nc.vector.tensor_copy(
                                v_sb[(b, jt)][0:rows, n2 * 6 : (n2 + 1) * 6, 0:D],
                                psv[0:rows, 0 : C // 2],
                            )
                        else:
                            nc.scalar.activation(
                                v_sb[(b, jt)][0:rows, n2 * 6 : (n2 + 1) * 6, 0:D],
                                psv[0:rows, 0 : C // 2],
                                AF.Copy,
                            )

            # ---- Phase A (by head-pair) interleaved with Phase C ----
            # Q weights host-scaled by SQ=256 (incl. softmax scale/temp),
            # K weights by SK=64; undone by the ACT scale here.
            def emit_proj_tile(mt):
                inv_s = (1.0 / 256.0) if mt < KT else (1.0 / 64.0)
                for nt in range(NT):
                    ps = psA.tile([128, 512], f32, tag="psA")
                    # psum cols in window-memory order [w0 w2 w1 w3]:
                    # offsets 0,128,256,325 (128-token windows first)
                    WMO = ((0, 128), (128, 128), (256, 69), (325, 69))
                    first = True
                    for p in range(KT // 2):
                        for w, (o, gl) in enumerate(WMO):
                            nc.tensor.matmul(
                                ps[:, o : o + gl],
                                wqkh_sb[:, mt, p, :, :],
                                xh_sb[:, nt, w, p, :, 0:gl],
                                start=first,
                                stop=False,
                                perf_mode=DR,
                            )
                            first = False
                    for k in range(KT):
                        for w, (o, gl) in enumerate(WMO):
                            nc.tensor.matmul(
                                ps[:, o : o + gl],
                                wqkx_sb[:, mt, k, :, :],
                                xx_sb[:, nt, w, k, :, 0:gl],
                                start=False,
                                stop=(k == KT - 1 and w == 3),
                                perf_mode=DR,
                            )
                    # permute [w0 w2 w1 w3] back to token order while
                    # converting psum->sbuf (strided dst); roughly 1 in 5
                    # tiles runs on DVE (tensor_scalar: scale then bias) to
                    # balance the ACT/DVE load in the attention region
                    dst = qk_sb[mt][:, nt * TN : (nt + 1) * TN].rearrange(
                        "p (g n) -> p g n", g=2
                    )
                    if (mt * NT + nt) % 4 == 0:
                        for sl_d, sl_s in (
                            ((0, 128), (0, 256)),
                            ((128, N), (256, 394)),
                        ):
                            nc.vector.tensor_scalar(
                                dst[:, :, sl_d[0] : sl_d[1]],
                                ps[:, sl_s[0] : sl_s[1]].rearrange(
                                    "p (w t) -> p w t", w=2
                                ),
                                inv_s,
                                bqk_sb[:, mt : mt + 1],
                                OP.mult,
                                OP.add,
                            )
                    else:
                        for sl_d, sl_s in (
                            ((0, 128), (0, 256)),
                            ((128, N), (256, 394)),
                        ):
                            nc.scalar.activation(
                                dst[:, :, sl_d[0] : sl_d[1]],
                                ps[:, sl_s[0] : sl_s[1]].rearrange(
                                    "p (w t) -> p w t", w=2
                                ),
                                AF.Identity,
                                bias=bqk_sb[:, mt : mt + 1],
                                scale=inv_s,
                            )

            # ---- Phase D emitted per nt-window as soon as the last
            # head-pair's batches 2nt..2nt+1 are done (fills the PE idle
            # during the attention tail); proj bias added on host ----
            def emit_d(nt):
                for mt in range(KT):
                    ps = psA.tile([128, 512], f32, tag="psA")
                    for k in range(KT):
                        nc.tensor.matmul(
                            ps[:, 0:TN],
                            wpj_sb[k][:, mt * 128 : (mt + 1) * 128],
                            attn_sb[k][:, nt * TN : (nt + 1) * TN],
                            start=(k == 0),
                            stop=(k == KT - 1),
                        )
                    ot = wp.tile([128, TN], f32, tag="ot", bufs=3)
                    nc.scalar.activation(ot[:], ps[:, 0:TN], AF.Copy)
                    nc.sync.dma_start(
                        out=outT[mt, :, nt * TN : (nt + 1) * TN], in_=ot[:]
                    )

            for hp in range(KT):
                if hp == 3:
                    for k in range(KT):
                        nc.sync.dma_start(out=wpj_sb[k][:], in_=wpj[k])
                for jt, rows in enumerate(JROWS):
                    nc.sync.dma_start(
                        out=bias_sb[(hp, jt)][:],
                        in_=bT[hp, jt * 128 : jt * 128 + rows, :],
                    )
                emit_proj_tile(hp)          # Q^T tile for this head pair
                emit_proj_tile(KT + hp)     # K^T tile
                for b in range(BPC):
                    # S^T for both heads of the pair in a 2-bank PSUM tile:
                    # bank dim = head (each matmul is its own accumulation
                    # group -- HW does not support two groups packed into one
                    # bank).  exp/e-mul then process both heads in one op.
                    e_tiles = []
                    for jt, rows in enumerate(JROWS):
                        ps = psC.tile([128, 2, 512], f32, tag="psC")
                        for hh in range(2):
                            base = 64 * hh
                            nc.tensor.matmul(
                                ps[0:rows, hh, 0:N],
                                qk_sb[KT + hp][
                                    base : base + 64,
                                    b * N + jt * 128 : b * N + jt * 128 + rows,
                                ],
                                qk_sb[hp][base : base + 64, b * N : (b + 1) * N],
                                start=True,
                                stop=True,
                            )
                        eu = wp.tile([128, 2, N], bf16, tag=f"eu{jt}", bufs=3)
                        nc.scalar.activation(
                            eu[0:rows, :, :], ps[0:rows, :, 0:N], AF.Exp
                        )
                        # multiplicative rel-pos bias (exp'd on host)
                        e = wp.tile([128, 2, N], bf16, tag=f"e{jt}", bufs=3)
                        nc.vector.tensor_mul(
                            e[0:rows, :, :],
                            eu[0:rows, :, :],
                            bias_sb[(hp, jt)][0:rows, :].rearrange(
                                "p (g n) -> p g n", g=2
                            ),
                        )
                        e_tiles.append(e)
                    # AV per head into its own bank of a 2-bank tile;
                    # lhsT = [V_h | 1] so row 64 = softmax denominators
                    po = psC.tile([128, 2, 512], f32, tag="psC")
                    for hh in range(2):
                        h = 2 * hp + hh
                        for jt, rows in enumerate(JROWS):
                            nc.tensor.matmul(
                                po[0 : D + 1, hh, 0:N],
                                v_sb[(b, jt)][0:rows, h, 0 : D + 1],
                                e_tiles[jt][0:rows, hh, :],
                                start=(jt == 0),
                                stop=(jt == 1),
                            )
                    # reciprocal of denominators (row 64, both heads) on DVE
                    # (GPSIMD cannot read PSUM), then broadcast to all
                    # partitions on GPSIMD (SBUF -> SBUF)
                    r2 = wp.tile([1, 2, N], bf16, tag="r2", bufs=3)
                    with nc.allow_low_precision(
                        reason="softmax denom reciprocal in bf16"
                    ):
                        nc.vector.reciprocal(r2[:, :, :], po[D : D + 1, :, 0:N])
                    rb = wp.tile([128, N2], bf16, tag="rb", bufs=3)
                    nc.gpsimd.partition_broadcast(rb[:, :], r2[:, :, :])
                    # divide = multiply by reciprocal; head1 partition-shifts
                    # to rows 64..127
                    nc.vector.tensor_mul(
                        attn_sb[hp][0:D, b * N : (b + 1) * N],
                        po[0:D, 0, 0:N],
                        rb[0:D, 0:N],
                    )
                    nc.vector.tensor_mul(
                        attn_sb[hp][D : 2 * D, b * N : (b + 1) * N],
                        po[0:D, 1, 0:N],
                        rb[D : 2 * D, N:N2],
                    )
                    if hp == KT - 1 and b % 2 == 1:
                        emit_d((b - 1) // 2)


    if finalize:
        nc.finalize()
    return nc


F8 = ml_dtypes.float8_e4m3fn
SQ, SK, SV = 256.0, 64.0, 64.0


def _split8(a):
    """Error-compensated fp8 pair: a ~= hi + lo, each e4m3."""
    hi = a.astype(F8)
    lo = (a - hi.astype(np.float32)).astype(F8)
    return hi, lo


def _ktiles(a, nf):
    """(768, nf) -> (128, KT, nf) partition-major k-tiles."""
    return np.ascontiguousarray(a.reshape(KT, 128, nf).transpose(1, 0, 2))


def _host_prep(x, qkv_w, qkv_b, proj_w, proj_b, rel_table, log_temp, rel_index):
    """Build the per-core input maps (host-side layout prep only)."""
    x = np.asarray(x, np.float32)
    qkv_w = np.asarray(qkv_w, np.float32)
    qkv_b = np.asarray(qkv_b, np.float32)
    proj_w = np.asarray(proj_w, np.float32)
    rel_table = np.asarray(rel_table, np.float32)
    log_temp = np.asarray(log_temp, np.float32)
    rel_index = np.asarray(rel_index)

    temp = np.log1p(np.exp(log_temp.astype(np.float64))).astype(np.float32)  # softplus
    alpha = (SCALE / temp).astype(np.float32)         # (H,) folded into q
    alpha_c = np.repeat(alpha, D)                     # (768,)

    # qk weights, host-scaled for fp8 range (SQ incl. alpha; SK plain),
    # split into hi/lo e4m3 pairs; hi-only and interleaved-cross layouts
    wqkT = qkv_w[0 : 2 * C].T.copy()                  # (768, 1536)
    wqkT[:, 0:C] *= alpha_c[None, :] * SQ
    wqkT[:, C : 2 * C] *= SK
    qhi, qlo = _split8(wqkT)
    qhi_t = _ktiles(qhi.astype(np.float32), 2 * C)
    qlo_t = _ktiles(qlo.astype(np.float32), 2 * C)
    # blocked pair-contiguous layouts for dual-fp8 Ldweights:
    # hi: [128, mt, pr, (hi_2p, hi_2p+1), 128]; x: [128, mt, k, (hi_k, lo_k), 128]
    wqk_hi_np = np.ascontiguousarray(
        qhi_t.reshape(128, KT // 2, 2, 2 * KT, 128).transpose(0, 3, 1, 2, 4)
    ).astype(F8)
    # cross weights LO-FIRST so the shared x cross buffer can stay HI-FIRST
    # (Phase B uses x as the weights side with (hi, lo) pairing)
    wqk_x_np = np.ascontiguousarray(
        np.stack([qlo_t, qhi_t], axis=2)
        .reshape(128, KT, 2, 2 * KT, 128)
        .transpose(0, 3, 1, 2, 4)
    ).astype(F8)

    # wv as fp8 hi/lo cross pairs, LO-FIRST (moving operand of Phase B),
    # host-scaled by SV out of the e4m3 subnormal range; the v_sb ones
    # column holds SV so the softmax divide cancels the scale exactly
    wvT = qkv_w[2 * C : 3 * C].T * SV                 # (768, 768)
    vhi, vlo = _split8(wvT)
    vhi_t = _ktiles(vhi.astype(np.float32), C)
    vlo_t = _ktiles(vlo.astype(np.float32), C)
    wvx = np.stack([vlo_t, vhi_t], axis=2)            # (128, KT, 2, C) lo-first
    wv_x_np = np.stack(
        [wvx[:, :, :, 0 : C // 2], wvx[:, :, :, C // 2 : C]], axis=0
    ).astype(F8)
    wpj_np = proj_w.T.reshape(KT, 128, C).astype(BF16)

    bq = qkv_b[0:C] * alpha_c
    bk = qkv_b[C : 2 * C]
    bqk_np = np.concatenate([bq, bk]).reshape(2 * KT, 128).T.copy().astype(np.float32)

    # multiplicative bias table: exp((relpos bias)/temp), diag -> 0, CLS -> 1,
    # transposed to (j, i)
    rpb = rel_table[rel_index]                        # (196, 196, H)
    bias = np.zeros((H, N, N), np.float32)
    bias[:, 1:, 1:] = rpb.transpose(2, 0, 1) / temp[:, None, None]
    ebias = np.exp(bias)
    idx = np.arange(1, N)
    ebias[:, idx, idx] = 0.0
    ebT = ebias.transpose(0, 2, 1)                    # (H, j, i)
    # paired layout: (KT, j, 2N) = heads 2hp | 2hp+1 side by side
    bT_np = (
        ebT.reshape(KT, 2, N, N).transpose(0, 2, 1, 3).reshape(KT, N, 2 * N)
    ).astype(BF16).copy()

    # window blocking: memory order [w0, w2, w1, w3], zero-padded to 128
    WOFF = (0, 197, 128, 325)
    WLEN = (128, 128, 69, 69)

    def _blocked(t, nunit):
        # t: (128, nunit, 2, T) -> (NT, 128, 4, nunit, 2, 128) padded
        out = np.zeros((NT, 128, 4, nunit, 2, 128), np.float32)
        for nt in range(NT):
            for w in range(4):
                o = nt * TN + WOFF[w]
                out[nt, :, w, :, :, 0 : WLEN[w]] = t[:, :, :, o : o + WLEN[w]]
        return out.astype(F8)

    in_maps = []
    for c in range(NCORES):
        xc = x[c * BPC : (c + 1) * BPC].reshape(T, C).T  # (768, T)
        xhi, xlo = _split8(xc)
        xhi_t = _ktiles(xhi.astype(np.float32), T)      # (128, KT, T)
        xlo_t = _ktiles(xlo.astype(np.float32), T)
        # hi pairs (hi_2p, hi_2p+1); cross pairs HI-FIRST (weights side of
        # Phase B; Phase A pairs its lo-first wqk cross against this)
        xhi_p = xhi_t.reshape(128, KT // 2, 2, T)
        xx_p = np.stack([xhi_t, xlo_t], axis=2)         # (128, KT, 2, T)
        x_hi_np = _blocked(xhi_p, KT // 2)
        x_x_np = _blocked(xx_p, KT)
        in_maps.append(
            {
                "x_hi": x_hi_np,
                "x_x": x_x_np,
                "wv_x": wv_x_np,
                "wqk_hi": wqk_hi_np,
                "wqk_x": wqk_x_np,
                "wpj": wpj_np,
                "bT": bT_np,
                "bqk": bqk_np,
            }
        )
    return in_maps


def kernel(**inputs) -> np.ndarray:
    global LAST_RESULTS
    from concourse.bass_utils import run_bass_kernel_spmd

    if "nc" not in _CACHE:
        _CACHE["nc"] = _build()
    nc = _CACHE["nc"]

    in_maps = _host_prep(**inputs)
    try:
        res = run_bass_kernel_spmd(
            nc, in_maps, core_ids=list(range(NCORES)), trace=TRACE
        )
    except ModuleNotFoundError:
        res = run_bass_kernel_spmd(
            nc, in_maps, core_ids=list(range(NCORES)), trace=False
        )
    LAST_RESULTS = res

    # v-bias rides through attention unchanged (rows of attn sum to 1), so
    # its proj image folds into the constant output bias added here
    proj_b = np.asarray(inputs["proj_b"], np.float32)
    proj_w = np.asarray(inputs["proj_w"], np.float32)
    bv = np.asarray(inputs["qkv_b"], np.float32)[2 * C : 3 * C]
    b_eff = proj_b + proj_w @ bv
    outs = []
    for c in range(NCORES):
        oT = np.asarray(res.results[c]["outT"], np.float32).reshape(C, T)
        outs.append(oT.T.reshape(BPC, N, C))
    out = np.concatenate(outs, axis=0) + b_eff[None, None, :]
    return out.astype(np.float32)
